# revision 22
# baseline (speedup 1.0000x reference)
"""Concatenation (additive/Bahdanau-style) attention Trainium2 kernel.

Math (per batch b):
    f = x @ W1[:H]          # [S, A]
    g = x @ W1[H:] + b1     # [S, A]
    scores[i, j] = w2 . tanh(f[i] + g[j]) + b2
    e = exp(scores) * (j < i)
    out[i] = sum_j e[i, j] x[j] / (sum_j e[i, j] + 1e-10)

Sharding: data-parallel over batch, one batch element per NeuronCore (B=8).

Key idea: the pairwise tanh is separable to high accuracy.  With
T=tanh(f), G=tanh(g) we have tanh(f+g) = (T+G)/(1+TG), a smooth 2-D
function whose Gaussian-weighted SVD decays exponentially.  We use a
skeleton (cross) approximation with basis functions tanh(.+node_k):

    tanh(f+g) ~= sum_{k,l} tanh(f+node_k) * M[k,l] * tanh(g+node_l)

(k=0 is a constant feature, tanh(arg+20)=1).  The fit matrix M is
input-independent (fit offline on a Gaussian-weighted grid; fitting is
done modulo additive functions of f, which cancel in the row softmax).
Then

    scores[i,j] ~= sum_{a,k} U[(a,k), i] * Vt[(a,k), j]

with U[(a,k), i] = tanh(f_ia + node_k) and
Vt[(a,k), j] = w2_a * sum_l M[k,l] tanh(g_ja + b1_a + node_l):
one PE matmul per 128-row j-supertile replaces the S^2*A/2 pairwise
tanh evaluations (60us of ACT time) entirely.

Per-core schedule:
  - replicate matmuls: lhsT columns (a,k) = W1f[:,a] (resp W1g) repeated
    per node slot -> PSUM [128/112, S]; one ACT tanh per 512-col piece
    with per-partition bias = node_k (U side; 20.0 for the const row)
    or node_l + b1_a (V side) -> fp16 features in SBUF.
  - fold matmul: block-diag FOLD[(a,l),(a,k)] = w2_a*M[k,l] -> PSUM,
    DVE-copied to fp16 Vt (lhsT of the score matmuls).
  - supertile g (j in [128g,128g+128), i in [128g, S)): score matmul
    contracting all 128 (a,k) rows -> PSUM [128, Lg]; ACT exp
    (bias=0; b2 cancels in softmax); strictly-upper fp16 mask on the
    diagonal 128-chunk (DVE) enforces j < i.
  - out: for each 128-row i-block ib, accumulate matmuls over
    supertiles g<=ib: lhsT = e_g[:, i-cols] (K=j), rhs = x_aug (x with
    a ones column) so the softmax denominator falls out of the same
    matmuls; then reciprocal+scale (DVE) and DMA out.
  - exp(g) is emitted after scores(g+1) so PE streams scores g+1 while
    ACT exponentiates supertile g.
"""

import numpy as np

import concourse.bass as bass
import concourse.tile as tile
from concourse import bacc, mybir
from concourse.bass_utils import run_bass_kernel_spmd

B, S, H, A = 8, 1024, 128, 16
NCORES = 8
XAUG_W = H + 4  # x plus a ones column, padded to 132 floats

FT = mybir.ActivationFunctionType
F32 = mybir.dt.float32
F16 = mybir.dt.float16  # fp16: same 1 col/cycle as bf16, 8x the mantissa

K = 7            # tanh nodes per hidden channel
KP1 = K + 1      # + one constant feature per channel
DU = A * KP1     # 128: score-matmul contraction dim (exactly fills PE)
DV = A * K       # 112 raw V rows
NODES = np.array([-2.3907, -1.2389, -0.4027, 0.0, 0.4027, 1.2389, 2.3907])


def _fit_M():
    """Weighted LSQ fit of tanh(f+g) in the tanh(.+node) skeleton basis.

    Fit is modulo additive functions of f (V-side basis and target are
    centered along g): those cancel in the softmax over j.  Input-
    independent; computed once at import (pure numpy, ~10ms).
    """
    n = 1101
    xg = np.linspace(-5.0, 5.0, n)
    Yg = np.tanh(xg[:, None] + xg[None, :])
    w = np.exp(-(xg ** 2) / 2.0)  # sigma=1 (actual f,g sigma ~0.71)
    w = w + 2e-3 * w.max()        # floor so corners stay sane
    w /= w.sum()
    sw = np.sqrt(w)
    Au = np.concatenate(
        [np.ones((n, 1)), np.tanh(xg[:, None] + NODES[None, :])], axis=1
    )
    Bv = np.tanh(NODES[None, :] + xg[:, None])
    Bc = Bv - (Bv * w[:, None]).sum(0, keepdims=True)
    Yc = Yg - (Yg * w[None, :]).sum(1, keepdims=True)

    def pinvr(Aw, r=1e-7):
        U_, S_, Vt_ = np.linalg.svd(Aw, full_matrices=False)
        return (Vt_.T * (S_ / (S_ ** 2 + r * S_[0] ** 2))) @ U_.T

    return pinvr(Au * sw[:, None]) @ (Yc * sw[:, None] * sw[None, :]) @ pinvr(
        Bc * sw[:, None]
    ).T  # [KP1, K]


M_FIT = _fit_M()


def _build_nc():
    nc = bacc.Bacc(None)

    # consts pack (f16): cols [0:DU+DV)=WW replicate weights,
    # [DU+DV : DU+DV+DU) = FOLD (rows 0:DV), last 128 = strictly-upper mask
    CP_W = DU + DV + DU + 128
    xT_d = nc.declare_dram_parameter("xT", [H, S], F16, isOutput=False)
    # x_aug pre-rearranged on host to [p, g, w]: row p holds j = 128g + p
    xaug_d = nc.declare_dram_parameter("x_aug", [128, 8, XAUG_W], F16, isOutput=False)
    cp_d = nc.declare_dram_parameter("CPACK", [128, CP_W], F16, isOutput=False)
    bias_d = nc.declare_dram_parameter("BIASM", [128, 4], F32, isOutput=False)
    out_d = nc.declare_dram_parameter("out", [S, H], F32, isOutput=True)

    with tile.TileContext(nc) as tc:
        with (
            tc.tile_pool(name="consts", bufs=1) as consts,
            tc.tile_pool(name="xa", bufs=1) as xapool,
            tc.tile_pool(name="e", bufs=1) as epool,
            tc.tile_pool(name="o", bufs=3) as opool,
            tc.tile_pool(name="psb", bufs=2, space="PSUM") as ps_big,
            tc.tile_pool(name="pss", bufs=1, space="PSUM") as ps_small,
        ):
            # ---- input loads: each DMA op blocks its engine's queue for the
            # transfer, so order by when the data is needed.  Both HW DGE
            # queues (SP, ACT); no gpsimd SWDGE (its multi-us drain blocks
            # dependents).  PE can start once CPACK + xT[:,0:512] land.
            cpack = consts.tile([128, CP_W], F16)
            nc.sync.dma_start(out=cpack, in_=cp_d[:, :])
            xT = consts.tile([H, S], F16)
            nc.scalar.dma_start(out=xT[:, 0:512], in_=xT_d[:, 0:512])
            nc.sync.dma_start(out=xT[:, 512:S], in_=xT_d[:, 512:S])
            biasm = consts.tile([128, 4], F32)
            nc.scalar.dma_start(out=biasm, in_=bias_d[:, :])
            xaug = xapool.tile([128, 8, XAUG_W], F16)
            nc.sync.dma_start(out=xaug[:, :, :], in_=xaug_d[:, :, :])

            ww = cpack[:, 0 : DU + DV]
            fold = cpack[0:DV, DU + DV : DU + DV + DU]
            maskf = cpack[:, DU + DV + DU : CP_W]
            ubias = biasm[0:DU, 0:1]
            vbias = biasm[0:DV, 1:2]
            zbias = biasm[:, 2:3]

            # ---- features.  V first (it feeds the longer fold+copy chain).
            # Emission note: a consumer's wait is served when the producing
            # engine's sequencer reaches the trailing sem-update, so a tanh
            # that (conservatively, tile-granular PSUM tracking) depends on
            # the side's LAST matmul piece gets its event promptly; pieces
            # emitted mm,mm,tanh,tanh pipeline best in practice.
            # raw: Vraw[(a,l), j] = tanh(g_ja + b1_a + node_l)
            psV = ps_big.tile([DV, S], F32, tag="big")
            Vraw = consts.tile([DV, S], F16)
            psU = ps_big.tile([DU, S], F32, tag="big")
            U = consts.tile([DU, S], F16)
            # V tanh split in two pieces (fold piece 0 starts earlier); U tanh
            # as one call over both banks (ACT reads may cross PSUM banks,
            # only matmul writes may not) — fewer instructions, less overhead
            for ps_t, sb_t, w_sl, bias, tanh_splits in (
                (psV, Vraw, slice(DU, DU + DV), vbias, 2),
                (psU, U, slice(0, DU), ubias, 1),
            ):
                for c in range(2):
                    sl = slice(512 * c, 512 * (c + 1))
                    nc.tensor.matmul(
                        out=ps_t[:, sl],
                        lhsT=ww[:, w_sl],
                        rhs=xT[:, sl],
                        start=True,
                        stop=True,
                    )
                step = S // tanh_splits
                for c in range(tanh_splits):
                    sl = slice(step * c, step * (c + 1))
                    nc.scalar.activation(
                        out=sb_t[:, sl], in_=ps_t[:, sl], func=FT.Tanh,
                        bias=bias, scale=1.0,
                    )

            # ---- fold: Vt[(a,k), j] = w2_a sum_l M[k,l] Vraw[(a,l), j]
            # (512-col PSUM pieces on the po1/po2 banks; the main loop's
            # first use of those banks comes well after the copies drain)
            Vt = consts.tile([DU, S], F16)
            for c in range(2):
                sl = slice(512 * c, 512 * (c + 1))
                psF = ps_small.tile([DU, 512], F32, tag=f"po{c + 1}",
                                    name=f"psF{c}")
                nc.tensor.matmul(
                    out=psF[:, :], lhsT=fold[:, :], rhs=Vraw[:, sl],
                    start=True, stop=True,
                )
                if c == 0:
                    # the g=0 score matmul only needs columns 0:128
                    nc.vector.tensor_copy(Vt[:, 0:128], psF[:, 0:128])
                    nc.vector.tensor_copy(Vt[:, 128:512], psF[:, 128:512])
                else:
                    nc.vector.tensor_copy(Vt[:, sl], psF[:, :])

            # ---- out-matmul bookkeeping (interleaved into the main loop;
            # 4 rotating PSUM tiles: ib and ib+4 share tag po{ib%4})
            po_tiles = {}
            next_term = {}  # ib -> next supertile index to accumulate
            active = []

            def activate_ib(ib):
                po_tiles[ib] = ps_small.tile(
                    [128, XAUG_W], F32, tag=f"po{ib % 4}", name=f"po_{ib}"
                )
                next_term[ib] = 0
                active.append(ib)

            def finish_ib(ib):
                po = po_tiles[ib]
                rec = opool.tile([128, 1], F32, tag="rec")
                if ib == 0:
                    # only row i=0 has an empty sum (reference adds 1e-10)
                    nc.vector.tensor_scalar_add(
                        out=rec, in0=po[:, H : H + 1], scalar1=1e-10
                    )
                    nc.vector.reciprocal(out=rec, in_=rec)
                else:
                    nc.vector.reciprocal(out=rec, in_=po[:, H : H + 1])
                osb = opool.tile([128, H], F32, tag="osb")
                nc.vector.tensor_scalar_mul(out=osb, in0=po[:, 0:H], scalar1=rec)
                # ib7's store rides the scalar queue (free once the last exp
                # is done) so the final two stores overlap; earlier stores
                # must NOT block ACT mid-loop
                q = nc.scalar if ib == 7 else nc.sync
                q.dma_start(out=out_d[ib * 128 : (ib + 1) * 128, :], in_=osb)
                active.remove(ib)
                if ib + 4 < 8:
                    activate_ib(ib + 4)

            def e_lhsT(g2, col0):
                # e columns [col0, col0+128) of supertile g2's e row-block
                for gc0, gc1, tile, toff in e_map[g2]:
                    if gc0 <= col0 < gc1:
                        return tile[:, toff + col0 : toff + col0 + 128]
                raise AssertionError

            def emit_out_terms(g):
                # out[i,:] = sum_j e[j,i]*x_aug[j]; accumulate terms whose
                # e-supertile is ready, for every ib with a live PSUM slot
                for ib in sorted(active):
                    while next_term[ib] <= min(ib, g):
                        g2 = next_term[ib]
                        nc.tensor.matmul(
                            out=po_tiles[ib][:, :],
                            lhsT=e_lhsT(g2, 128 * (ib - g2)),
                            rhs=xaug[:, g2, :],
                            start=(g2 == 0),
                            stop=(g2 == ib),
                        )
                        next_term[ib] += 1
                    if next_term[ib] > ib:
                        finish_ib(ib)

            for ib in range(4):
                activate_ib(ib)

            # ---- main loop over supertiles (128 j's each).  The exp for
            # chunk k is emitted after the score matmuls of chunk k+1 so ACT
            # exponentiates chunk k while PE streams chunk k+1.  Supertile 0
            # is split into two independent [128, 512] PSUM tiles so its
            # first exp starts earlier; supertiles (4,5) and (6,7) share one
            # PSUM tile + one exp call (ACT reads may cross banks) to cut
            # per-ACTIVATE overhead.
            # chunk: list of (g, gc0, gc1) score segments sharing one exp
            chunks = [[(0, 0, 512)], [(0, 512, 1024)]] + [
                [(g, 0, S - 128 * g)] for g in range(1, 8)
            ]
            e_map = {g: [] for g in range(8)}  # g -> [(gc0, gc1, tile, toff)]

            def emit_chunk(segs, ps):
                W = sum(c1 - c0 for _, c0, c1 in segs)
                tag = "e" + "_".join(f"{g}c{c0}" for g, c0, _ in segs)
                e = epool.tile([128, W], F16, tag=tag, name=tag)
                off = 0
                for g, c0, c1 in segs:
                    e_map[g].append((c0, c1, e, off - c0))
                    off += c1 - c0
                nc.scalar.activation(
                    out=e[:, :], in_=ps[:, 0:W], func=FT.Exp, bias=zbias,
                    scale=1.0,
                )
                for g, c0, c1 in segs:
                    if c0 == 0:  # this segment holds g's diagonal 128 cols
                        lhs = e_lhsT(g, 0)
                        nc.vector.tensor_mul(lhs, lhs, maskf)
                for g, c0, c1 in segs:
                    if c1 == S - 128 * g:  # g complete
                        emit_out_terms(g)

            pending = None
            for segs in chunks:
                W = sum(c1 - c0 for _, c0, c1 in segs)
                ps = ps_big.tile([128, W], F32, tag="big")
                off = 0
                for g, c0, c1 in segs:
                    bounds = list(range(c0, c1, 512)) + [c1]
                    for b0, b1 in zip(bounds[:-1], bounds[1:]):
                        nc.tensor.matmul(
                            out=ps[:, off + b0 - c0 : off + b1 - c0],
                            lhsT=Vt[:, 128 * g : 128 * (g + 1)],
                            rhs=U[:, 128 * g + b0 : 128 * g + b1],
                            start=True,
                            stop=True,
                        )
                    off += c1 - c0
                if pending is not None:
                    emit_chunk(*pending)
                pending = (segs, ps)
            emit_chunk(*pending)

    nc.compile()
    return nc


_NC_CACHE = None


def _get_nc():
    global _NC_CACHE
    if _NC_CACHE is None:
        _NC_CACHE = _build_nc()
    return _NC_CACHE


def _host_prep(x, W1, b1, w2, b2):
    """Build the per-core input maps (small derived tensors + shards)."""
    x = np.asarray(x, dtype=np.float32)
    W1 = np.asarray(W1, dtype=np.float32)
    b1 = np.asarray(b1, dtype=np.float32).reshape(-1)
    w2 = np.asarray(w2, dtype=np.float32).reshape(-1)

    W1f, W1g = W1[:H], W1[H:]  # [H, A] each
    CP_W = DU + DV + DU + 128
    CPACK = np.zeros((128, CP_W), dtype=np.float16)
    BIASM = np.zeros((128, 4), dtype=np.float32)
    for a in range(A):
        CPACK[:, a * KP1 : (a + 1) * KP1] = W1f[:, a : a + 1]
        CPACK[:, DU + a * K : DU + (a + 1) * K] = W1g[:, a : a + 1]
        CPACK[a * K : (a + 1) * K, DU + DV + a * KP1 : DU + DV + (a + 1) * KP1] = (
            w2[a] * M_FIT.T
        )
        BIASM[a * KP1, 0] = 20.0  # const feature: tanh(f+20) == 1
        BIASM[a * KP1 + 1 : (a + 1) * KP1, 0] = NODES
        BIASM[a * K : (a + 1) * K, 1] = NODES + b1[a]
    p = np.arange(128)
    CPACK[:, DU + DV + DU :] = (p[:, None] < p[None, :]).astype(np.float16)

    shared = {"CPACK": CPACK, "BIASM": BIASM}
    in_maps = []
    for c in range(NCORES):
        xb = x[c]  # [S, H]
        x_aug = np.zeros((S, XAUG_W), dtype=np.float16)
        x_aug[:, :H] = xb
        x_aug[:, H] = 1.0
        m = dict(shared)
        # [p, g, w]: row p holds j = 128g + p (device reads it contiguously)
        m["x_aug"] = np.ascontiguousarray(
            x_aug.reshape(8, 128, XAUG_W).transpose(1, 0, 2)
        )
        m["xT"] = np.ascontiguousarray(xb.T).astype(np.float16)
        in_maps.append(m)
    return in_maps


def kernel(x, W1, b1, w2, b2, _trace=False):
    nc = _get_nc()
    in_maps = _host_prep(x, W1, b1, w2, b2)
    res = run_bass_kernel_spmd(nc, in_maps, list(range(NCORES)), trace=_trace)
    out = np.stack([np.asarray(res.results[c]["out"]) for c in range(NCORES)])
    if _trace:
        kernel.last_exec_time_ns = res.exec_time_ns
        kernel.last_profile = res.profile_json
    return out
